# revision 1
# baseline (speedup 1.0000x reference)
"""Trainium2 Bass kernel for additive (Bahdanau-style) attention.

Reference computation (fp32):
    enc    = encoder_output.transpose(1, 0, 2)            # [B, S, F]
    concat = [enc, broadcast(decoder_hidden)]             # [B, S, F+D]
    h      = tanh(concat @ W1.T + b1)                     # [B, S, D]
    scores = h @ W2.T + b2                                # [B, S, 1]
    alpha  = softmax(scores, axis=S)
    out    = einsum('bs,bsf->bf', alpha[..., 0], enc)[:, None, :]   # [B, 1, F]

Sharding: data-parallel over batch — 8 NeuronCores x 4 batches each.
Weights are tiny and replicated.

Per-core device program (all heavy tensors fp16, fp32 accumulation):
  1. dec_proj[d, b] = W1[:, F:] @ decoder_hidden[b] + b1     (PE, once)
  2. hT[d, s] = enc[f, s-block] contracted with W1[:, :F]    (PE, fp16, PSUM fp32)
  3. tanh via ScalarE with per-partition bias = dec_proj col (PSUM -> SBUF fp16)
  4. scores[1, s] = w2 . tanh_h  (PE, lhsT = w2 column)
  5. e = exp(scores - SHIFT), Z += sum(e)                    (ScalarE accum_out)
  6. broadcast e to 128 partitions via ones-matmul           (PE)
  7. acc[f-tile] = sum_s enc[f, s] * e_bc[s]   (VectorE scalar_tensor_tensor
     with accum_out: fused multiply + free-dim reduce)
  8. out = acc * (1/Z)                                       (VectorE)
  For the last local batch (PE_WS_LAST) steps 6-8 instead run on the
  tensor engine: scores^T matmuls put exp() S-partitioned, then
  out[1, F] = e^T . enc_sf with a host-provided [S-part, F-free] enc
  copy — the VectorE weighted sum is the critical engine (HW fp16
  matmuls at N=512 stream at ~110 ns, 2x the cost model), so shifting
  one batch of it to the PE rebalances the engines.

The encoder tensor is laid out host-side as [B, NFT, 128, S] fp16 with the
feature index interleaved as f = p * NFT + ft so that the final [128, NFT]
accumulator DMAs to DRAM with contiguous 4*NFT-byte runs per partition.
softmax uses a constant shift instead of the max (mathematically identical;
scores are O(1) for this model so exp() cannot overflow fp32, and the fp16
e-values stay in range via EXP_SHIFT).
"""

import json

import numpy as np

import concourse.bass as bass
import concourse.mybir as mybir
import concourse.tile as tile
from concourse.bass_utils import run_bass_kernel_spmd


def _split_multi_waits(nc: bass.Bass) -> None:
    """Walrus in this container rejects instructions with >1 sync-wait
    command ("Too many sync wait commands", CoreV3GenImpl setupSyncWait).
    Tile's kernel-tail drain carries one wait per outstanding processor.
    Split the extras into standalone single-wait EventSemaphore
    instructions placed immediately before, on the same engine."""
    d = json.loads(mybir.module_to_json_string(nc.m))
    n_new = 0
    for f in d.get("functions", []):
        for bb in f.get("blocks", []):
            new_insts = []
            for ins in bb.get("instructions", []):
                si = ins.get("sync_info") or {}
                ow = si.get("on_wait") or []
                if len(ow) > 1:
                    for w in ow[:-1]:
                        n_new += 1
                        new_insts.append({
                            "debug": ins.get("debug", 0),
                            "engine": ins["engine"],
                            "ins": [],
                            "outs": [],
                            "name": f"{ins['name']}-swait{n_new}",
                            "opcode": "EventSemaphore",
                            "sync_info": {"on_update": [], "on_wait": [w]},
                        })
                    si["on_wait"] = [ow[-1]]
                new_insts.append(ins)
            bb["instructions"] = new_insts
    if n_new:
        fixed = json.dumps(d).encode()
        nc.to_json_bytes = lambda: fixed  # type: ignore[method-assign]

F16 = mybir.dt.float16
F32 = mybir.dt.float32

N_CORES = 8
EXP_SHIFT = -3.0
# Offload the last local batch's softmax-weighted sum to the tensor engine
# (frees the critical VectorE; needs a second [S-part, F-free] enc layout).
PE_WS_LAST = True


class Cfg:
    def __init__(self, S=1024, F=1024, D=512, DEC=512, BPC=4):
        self.S, self.F, self.D, self.DEC, self.BPC = S, F, D, DEC, BPC
        self.NFT = F // 128      # feature tiles (partition dim of enc layout)
        self.NM = D // 128       # M-tiles of hT / K-tiles of the scores matmul
        self.NKD = DEC // 128    # K-tiles of the dec_proj matmul
        self.SBLK = min(512, S)  # PSUM free-dim block
        self.NSB = S // self.SBLK
        assert F % 128 == 0 and D % 128 == 0 and DEC % 128 == 0
        assert S % self.SBLK == 0


FULL = Cfg()


def build_bass(cfg: Cfg, repeat: int = 1) -> bass.Bass:
    S, F, D, DEC, BPC = cfg.S, cfg.F, cfg.D, cfg.DEC, cfg.BPC
    NFT, NM, NKD, SBLK, NSB = cfg.NFT, cfg.NM, cfg.NKD, cfg.SBLK, cfg.NSB

    nc = bass.Bass()

    enc_in = nc.dram_tensor("enc_in", [BPC, NFT, 128, S], F16, kind="ExternalInput")
    w1e_in = nc.dram_tensor("w1e", [NFT, 128, D], F16, kind="ExternalInput")
    w1d_in = nc.dram_tensor("w1d", [NKD, 128, D], F16, kind="ExternalInput")
    b1_in = nc.dram_tensor("b1r", [1, D], F16, kind="ExternalInput")
    w2_in = nc.dram_tensor("w2t", [NM, 128], F16, kind="ExternalInput")
    dec_in = nc.dram_tensor("dect", [NKD, 128, BPC], F16, kind="ExternalInput")
    out_dram = nc.dram_tensor("out", [BPC, 128, NFT], F32, kind="ExternalOutput")
    if PE_WS_LAST:
        sf_in = nc.dram_tensor("enc_sf", [S // 128, 128, F], F16,
                               kind="ExternalInput")
        out2_dram = nc.dram_tensor("out2", [1, F], F32, kind="ExternalOutput")

    Tanh = mybir.ActivationFunctionType.Tanh
    Exp = mybir.ActivationFunctionType.Exp
    AX = mybir.AxisListType.X
    MUL = mybir.AluOpType.mult
    ADD = mybir.AluOpType.add

    with tile.TileContext(nc) as tc:
        with (
            tc.tile_pool(name="consts", bufs=1) as consts,
            tc.tile_pool(name="encp", bufs=3 * NFT) as encp,
            tc.tile_pool(name="thp", bufs=2 * NM + 2) as thp,
            tc.tile_pool(name="ep", bufs=3) as ep_pool,
            tc.tile_pool(name="zp", bufs=3) as zp,
            tc.tile_pool(name="scrp", bufs=4) as scrp,
            tc.tile_pool(name="sfp", bufs=2) as sfp,
            tc.tile_pool(name="etp", bufs=2) as etp,
            tc.tile_pool(name="accp", bufs=3) as accp,
            tc.tile_pool(name="outp", bufs=3) as outp_pool,
            tc.tile_pool(name="hps", bufs=4, space="PSUM") as hps,
            tc.tile_pool(name="scps", bufs=2, space="PSUM") as scps,
            tc.tile_pool(name="miscps", bufs=2, space="PSUM") as miscps,
        ):
            # ---- constants (small dec-path operands first: PE work sooner) ----
            w1d_sb = consts.tile([128, NKD, D], F16)
            nc.sync.dma_start(out=w1d_sb, in_=w1d_in.rearrange("a p d -> p a d"))
            b1_sb = consts.tile([1, D], F16)
            nc.sync.dma_start(out=b1_sb, in_=b1_in[:, :])
            w2_sb = consts.tile([128, NM], F16)
            nc.sync.dma_start(out=w2_sb, in_=w2_in.rearrange("a p -> p a"))
            dect_sb = consts.tile([128, NKD, BPC], F16)
            nc.sync.dma_start(out=dect_sb, in_=dec_in.rearrange("a p b -> p a b"))
            # w1e chunks interleaved with batch-0 enc tiles in queue order:
            # each arriving (weights, enc) pair feeds the next 4 matmuls of
            # the kt-outer first block, so the PE computes during the
            # initial streaming instead of waiting for the full 3.6 MB
            w1e_sb = consts.tile([128, NFT, D], F16)
            enc0_tiles = []
            for kt in range(NFT):
                nc.sync.dma_start(out=w1e_sb[:, kt, :], in_=w1e_in[kt])
                et0 = encp.tile([128, S], F16, tag="enc_t", name="enc_t")
                nc.sync.dma_start(out=et0, in_=enc_in[0, kt])
                enc0_tiles.append(et0)

            onesb = consts.tile([1, BPC], F16)
            nc.vector.memset(onesb, 1.0)
            ones16 = consts.tile([1, 128], F16)
            nc.vector.memset(ones16, 1.0)
            ones32 = consts.tile([1, 128], F32)
            nc.vector.memset(ones32, 1.0)
            shift_sb = consts.tile([1, 1], F32)
            nc.vector.memset(shift_sb, EXP_SHIFT)
            if PE_WS_LAST:
                shift128 = consts.tile([128, 1], F32)
                nc.vector.memset(shift128, EXP_SHIFT)

            # ---- dec_proj^T + b1:  [128 (d%), NM * BPC] fp32 ----
            dec_sb = consts.tile([128, NM, BPC], F32)
            for m in range(NM):
                dp = miscps.tile([128, BPC], F32, tag="mps")
                for kd in range(NKD):
                    nc.tensor.matmul(
                        dp,
                        lhsT=w1d_sb[:, kd, m * 128:(m + 1) * 128],
                        rhs=dect_sb[:, kd, :],
                        start=(kd == 0),
                        stop=False,
                    )
                nc.tensor.matmul(
                    dp,
                    lhsT=b1_sb[0:1, m * 128:(m + 1) * 128],
                    rhs=onesb,
                    start=False,
                    stop=True,
                )
                nc.scalar.copy(dec_sb[:, m, :], dp)

            # ---- main loop over local batches ----
            for iter_i, b in enumerate(i % BPC for i in range(repeat * BPC)):
                if iter_i == 0:
                    # first batch: tiles already DMA'd, interleaved with w1e
                    enc_tiles = enc0_tiles
                else:
                    enc_tiles = []
                    for ft in range(NFT):
                        et = encp.tile([128, S], F16, tag="enc_t")
                        nc.sync.dma_start(out=et, in_=enc_in[b, ft])
                        enc_tiles.append(et)

                # the final S-block of the very last iteration is split in
                # half: the closing DVE weighted-sum chain (which nothing can
                # overlap) runs on half as many elements
                blocks = [(i * SBLK, SBLK) for i in range(NSB)]

                pe_ws = PE_WS_LAST and b == BPC - 1
                if pe_ws:
                    sf_sb = sfp.tile([128, S // 128, F], F16, tag="sf")
                    nc.sync.dma_start(
                        out=sf_sb, in_=sf_in.rearrange("a p f -> p a f")
                    )
                    eT_sb = etp.tile([128, S // 128], F16, tag="eT")

                e16 = ep_pool.tile([1, S], F16)
                zacc = zp.tile([1, len(blocks)], F32, tag="zacc")
                acc2 = accp.tile([128, len(blocks), NFT], F32)

                first_block = iter_i == 0
                for sb, (boff, bsz) in enumerate(blocks):
                    sl = slice(boff, boff + bsz)
                    th_tiles = []
                    if first_block and sb == 0:
                        # kt-outer: every arriving enc tile feeds 4 matmuls,
                        # so the PE computes while the first tiles stream in
                        hp_list = [hps.tile([128, bsz], F32, tag="hp", name="hp")
                                   for _ in range(NM)]
                        for kt in range(NFT):
                            for m in range(NM):
                                nc.tensor.matmul(
                                    hp_list[m],
                                    lhsT=w1e_sb[:, kt, m * 128:(m + 1) * 128],
                                    rhs=enc_tiles[kt][:, sl],
                                    start=(kt == 0),
                                    stop=(kt == NFT - 1),
                                )
                        for m in range(NM):
                            th = thp.tile([128, bsz], F16, tag="th")
                            nc.scalar.activation(
                                th, hp_list[m], Tanh,
                                bias=dec_sb[:, m, b:b + 1], scale=1.0,
                            )
                            th_tiles.append(th)
                    else:
                        for m in range(NM):
                            hp = hps.tile([128, bsz], F32, tag="hp")
                            for kt in range(NFT):
                                nc.tensor.matmul(
                                    hp,
                                    lhsT=w1e_sb[:, kt, m * 128:(m + 1) * 128],
                                    rhs=enc_tiles[kt][:, sl],
                                    start=(kt == 0),
                                    stop=(kt == NFT - 1),
                                )
                            th = thp.tile([128, bsz], F16, tag="th")
                            nc.scalar.activation(
                                th, hp, Tanh, bias=dec_sb[:, m, b:b + 1], scale=1.0
                            )
                            th_tiles.append(th)

                    sc = scps.tile([1, bsz], F32)
                    for m in range(NM):
                        nc.tensor.matmul(
                            sc,
                            lhsT=w2_sb[:, m:m + 1],
                            rhs=th_tiles[m],
                            start=(m == 0),
                            stop=(m == NM - 1),
                        )
                    # e = exp(scores + EXP_SHIFT); zacc[sb] = sum(e)
                    nc.scalar.activation(
                        e16[0:1, sl], sc, Exp,
                        bias=shift_sb[0:1, 0:1],
                        accum_out=zacc[0:1, sb:sb + 1],
                    )
                    if pe_ws:
                        # scores^T per 128-col chunk (lhsT = tanh slices) so
                        # exp lands S-partitioned for the PE weighted sum
                        for c in range(bsz // 128):
                            scT = miscps.tile([128, 1], F32, tag="mps",
                                              name="scT")
                            for kd in range(NM):
                                nc.tensor.matmul(
                                    scT,
                                    lhsT=th_tiles[kd][:, c * 128:(c + 1) * 128],
                                    rhs=w2_sb[:, kd:kd + 1],
                                    start=(kd == 0),
                                    stop=(kd == NM - 1),
                                )
                            ci = boff // 128 + c
                            nc.scalar.activation(
                                eT_sb[:, ci:ci + 1], scT, Exp,
                                bias=shift128[:, 0:1],
                            )
                        continue
                    # broadcast e-chunk to all partitions (ones-matmul on PE);
                    # the weighted sum reads it straight from PSUM.
                    ebc = miscps.tile([128, bsz], F32, tag="mps")
                    nc.tensor.matmul(
                        ebc, lhsT=ones16, rhs=e16[0:1, sl], start=True, stop=True
                    )
                    # fused multiply + free-dim reduce per feature tile
                    for ft in range(NFT):
                        scr = scrp.tile([128, bsz], F16, tag="scr")
                        nc.vector.scalar_tensor_tensor(
                            out=scr,
                            in0=enc_tiles[ft][:, sl],
                            scalar=1.0,
                            in1=ebc,
                            op0=MUL,
                            op1=MUL,
                            accum_out=acc2[:, sb, ft:ft + 1],
                        )

                z = zp.tile([1, 1], F32, tag="z")
                nc.vector.tensor_reduce(z, zacc, axis=AX, op=ADD)
                invz = zp.tile([1, 1], F32, tag="invz")
                nc.vector.reciprocal(invz, z)

                if pe_ws:
                    # weighted sum on PE: out[1, F] = e^T . enc_sf, then 1/Z
                    o1 = outp_pool.tile([1, F], F32, tag="o1")
                    for fo in range(0, F, 512):
                        w = min(512, F - fo)
                        aps = hps.tile([1, w], F32, tag="hp", name="aps")
                        for kt in range(S // 128):
                            nc.tensor.matmul(
                                aps,
                                lhsT=eT_sb[:, kt:kt + 1],
                                rhs=sf_sb[:, kt, fo:fo + w],
                                start=(kt == 0),
                                stop=(kt == S // 128 - 1),
                            )
                        nc.scalar.activation(
                            o1[0:1, fo:fo + w], aps,
                            mybir.ActivationFunctionType.Copy,
                            scale=invz[0:1, 0:1],
                        )
                    nc.sync.dma_start(out=out2_dram[0:1, :], in_=o1)
                    continue

                izp = scps.tile([128, 1], F32, tag="sc")
                nc.tensor.matmul(izp, lhsT=ones32, rhs=invz, start=True, stop=True)
                izb = zp.tile([128, 1], F32, tag="izb")
                nc.scalar.copy(izb, izp)

                o = outp_pool.tile([128, NFT], F32)
                nblk = len(blocks)
                if nblk == 1:
                    nc.vector.tensor_scalar_mul(o, acc2[:, 0, :], izb)
                else:
                    osum = outp_pool.tile([128, NFT], F32, tag="osum")
                    nc.vector.tensor_add(osum, acc2[:, 0, :], acc2[:, 1, :])
                    for sb in range(2, nblk):
                        nc.vector.tensor_add(osum, osum, acc2[:, sb, :])
                    nc.vector.tensor_scalar_mul(o, osum, izb)
                nc.sync.dma_start(out=out_dram[b], in_=o)

    _split_multi_waits(nc)
    return nc


def prep_inputs(encoder_output, decoder_hidden, W1, b1, W2, cfg: Cfg):
    """Host-side sharding + layout. Returns per-core input maps."""
    S, F, D, DEC, BPC = cfg.S, cfg.F, cfg.D, cfg.DEC, cfg.BPC
    NFT, NM, NKD = cfg.NFT, cfg.NM, cfg.NKD
    n_cores = encoder_output.shape[1] // BPC

    # enc [S, B, F] -> [B, F, S] -> f = p*NFT + ft -> [B, NFT, 128, S] fp16
    enc_t = np.ascontiguousarray(encoder_output.transpose(1, 2, 0))
    enc_l = np.ascontiguousarray(
        enc_t.reshape(-1, 128, NFT, S).transpose(0, 2, 1, 3)
    ).astype(np.float16)

    w1e_t = W1[:, :F].T.astype(np.float32)  # [F, D]
    w1e = np.ascontiguousarray(
        w1e_t.reshape(128, NFT, D).transpose(1, 0, 2)
    ).astype(np.float16)

    w1d_t = W1[:, F:].T.astype(np.float32)  # [DEC, D]
    w1d = np.ascontiguousarray(w1d_t.reshape(NKD, 128, D)).astype(np.float16)

    b1r = b1.reshape(1, D).astype(np.float16)
    w2t = W2.reshape(D).reshape(NM, 128).astype(np.float16)
    dect_full = decoder_hidden.T.reshape(NKD, 128, -1).astype(np.float16)  # [.., B]

    in_maps = []
    for c in range(n_cores):
        bs = slice(c * BPC, (c + 1) * BPC)
        m = {
            "enc_in": enc_l[bs],
            "w1e": w1e,
            "w1d": w1d,
            "b1r": b1r,
            "w2t": w2t,
            "dect": np.ascontiguousarray(dect_full[:, :, bs]),
        }
        if PE_WS_LAST:
            sf = np.ascontiguousarray(
                encoder_output[:, c * BPC + BPC - 1, :]
            ).astype(np.float16)
            m["enc_sf"] = sf.reshape(S // 128, 128, F)
        in_maps.append(m)
    return in_maps


def assemble_core(r, cfg: Cfg):
    """Per-core output rows [BPC, F] from the result map."""
    out = r["out"].reshape(cfg.BPC, cfg.F).copy()
    if PE_WS_LAST:
        out[cfg.BPC - 1] = r["out2"][0]
    return out


def kernel(encoder_output, decoder_hidden, W1, b1, W2, b2):
    """Full inputs in, full output out. b2 cancels in the softmax."""
    encoder_output = np.asarray(encoder_output, dtype=np.float32)
    decoder_hidden = np.asarray(decoder_hidden, dtype=np.float32)
    W1 = np.asarray(W1, dtype=np.float32)
    b1 = np.asarray(b1, dtype=np.float32)
    W2 = np.asarray(W2, dtype=np.float32)

    cfg = FULL
    nc = build_bass(cfg)
    in_maps = prep_inputs(encoder_output, decoder_hidden, W1, b1, W2, cfg)
    res = run_bass_kernel_spmd(nc, in_maps, list(range(N_CORES)))
    out = np.concatenate(
        [assemble_core(r, cfg) for r in res.results], axis=0
    )
    return out[:, None, :].astype(np.float32)


if __name__ == "__main__":
    import reference

    inputs = reference.setup_inputs()
    expected = np.asarray(reference.reference(**inputs))
    actual = kernel(**{k: np.asarray(v) for k, v in inputs.items()})
    err = np.abs(actual - expected).max() / np.abs(expected).max()
    print("Relative error:", err)



# revision 2
# speedup vs baseline: 13.5898x; 13.5898x over previous
"""Trainium2 Bass kernel for additive (Bahdanau-style) attention — v3.

Reference computation (fp32):
    enc    = encoder_output.transpose(1, 0, 2)            # [B, S, F]
    concat = [enc, broadcast(decoder_hidden)]             # [B, S, F+D]
    h      = tanh(concat @ W1.T + b1)                     # [B, S, D]
    scores = h @ W2.T + b2                                # [B, S, 1]
    alpha  = softmax(scores, axis=S)
    out    = einsum('bs,bsf->bf', alpha[..., 0], enc)[:, None, :]   # [B, 1, F]

Sharding: data-parallel over batch — 8 NeuronCores x 4 batches each.
Weights are tiny and replicated.

v3 structure: the per-block work is split into pipeline stages emitted with
explicit lag so no engine's program order makes it wait on a slower engine:
  slot k (block granularity):  hp-matmuls(k) [PE] + tanh(k) [Act]
  then A(k-1): scores matmul [PE] + exp/accum [Act]
  then B(k-2): e-broadcast ones-matmul [PE] + PSUM->SBUF fp16 copy [Act]
  batch tail W: full-S fused multiply+reduce per feature tile [DVE],
  softmax normalizer, 1/Z scale and the output DMA — emitted ~2 slots into
  the next batch so the DVE weighted sum of batch i overlaps batch i+1's
  PE/Act work.
softmax uses a constant shift instead of the max (mathematically identical;
scores are O(1) for this model so exp() cannot overflow fp32, and the fp16
e-values stay in range via EXP_SHIFT).
"""

import json
from collections import deque

import numpy as np

import concourse.bass as bass
import concourse.mybir as mybir
import concourse.tile as tile
from concourse.bass_utils import run_bass_kernel_spmd


def _split_multi_waits(nc: bass.Bass) -> None:
    """Walrus in this container rejects instructions with >1 sync-wait
    command ("Too many sync wait commands", CoreV3GenImpl setupSyncWait).
    Tile's kernel-tail drain carries one wait per outstanding processor.
    Split the extras into standalone single-wait EventSemaphore
    instructions placed immediately before, on the same engine."""
    d = json.loads(mybir.module_to_json_string(nc.m))
    n_new = 0
    for f in d.get("functions", []):
        for bb in f.get("blocks", []):
            new_insts = []
            for ins in bb.get("instructions", []):
                si = ins.get("sync_info") or {}
                ow = si.get("on_wait") or []
                if len(ow) > 1:
                    for w in ow[:-1]:
                        n_new += 1
                        new_insts.append({
                            "debug": ins.get("debug", 0),
                            "engine": ins["engine"],
                            "ins": [],
                            "outs": [],
                            "name": f"{ins['name']}-swait{n_new}",
                            "opcode": "EventSemaphore",
                            "sync_info": {"on_update": [], "on_wait": [w]},
                        })
                    si["on_wait"] = [ow[-1]]
                new_insts.append(ins)
            bb["instructions"] = new_insts
    if n_new:
        fixed = json.dumps(d).encode()
        nc.to_json_bytes = lambda: fixed  # type: ignore[method-assign]

F16 = mybir.dt.float16
F32 = mybir.dt.float32

N_CORES = 8
EXP_SHIFT = -3.0
# Pipeline depth in deferred stages (A/B/W closures pending emission).
DEPTH = 3


class Cfg:
    def __init__(self, S=1024, F=1024, D=512, DEC=512, BPC=4, SBLK=512):
        self.S, self.F, self.D, self.DEC, self.BPC = S, F, D, DEC, BPC
        self.NFT = F // 128      # feature tiles (partition dim of enc layout)
        self.NM = D // 128       # M-tiles of hT / K-tiles of the scores matmul
        self.NKD = DEC // 128    # K-tiles of the dec_proj matmul
        self.SBLK = min(SBLK, S)  # PSUM free-dim block
        self.NSB = S // self.SBLK
        assert F % 128 == 0 and D % 128 == 0 and DEC % 128 == 0
        assert S % self.SBLK == 0


FULL = Cfg()


def build_bass(cfg: Cfg, repeat: int = 1, abl: str | None = None) -> bass.Bass:
    """abl: ablation mode for bottleneck localization (None = full kernel).
    "nows"    — skip the DVE weighted sum (out is wrong, timing only)
    "nosc"    — additionally skip scores/exp/e-broadcast stages
    "dmaonly" — only the enc DMA stream + a trivial consumer
    """
    S, F, D, DEC, BPC = cfg.S, cfg.F, cfg.D, cfg.DEC, cfg.BPC
    NFT, NM, NKD, SBLK, NSB = cfg.NFT, cfg.NM, cfg.NKD, cfg.SBLK, cfg.NSB

    nc = bass.Bass()

    enc_in = nc.dram_tensor("enc_in", [BPC, NFT, 128, S], F16, kind="ExternalInput")
    w1e_in = nc.dram_tensor("w1e", [NFT, 128, D], F16, kind="ExternalInput")
    w1d_in = nc.dram_tensor("w1d", [NKD, 128, D], F16, kind="ExternalInput")
    b1_in = nc.dram_tensor("b1r", [1, D], F16, kind="ExternalInput")
    w2_in = nc.dram_tensor("w2t", [NM, 128], F16, kind="ExternalInput")
    dec_in = nc.dram_tensor("dect", [NKD, 128, BPC], F16, kind="ExternalInput")
    out_dram = nc.dram_tensor("out", [BPC, 128, NFT], F32, kind="ExternalOutput")

    Tanh = mybir.ActivationFunctionType.Tanh
    Exp = mybir.ActivationFunctionType.Exp
    AX = mybir.AxisListType.X
    MUL = mybir.AluOpType.mult
    ADD = mybir.AluOpType.add

    with tile.TileContext(nc) as tc:
        with (
            tc.tile_pool(name="consts", bufs=1) as consts,
            tc.tile_pool(name="encp", bufs=3 * NFT) as encp,
            tc.tile_pool(name="thp", bufs=2 * NM + 2) as thp,
            tc.tile_pool(name="ep", bufs=3) as ep_pool,
            tc.tile_pool(name="zp", bufs=3) as zp,
            tc.tile_pool(name="ebcp", bufs=3) as ebcp,
            tc.tile_pool(name="scrp", bufs=6) as scrp,
            tc.tile_pool(name="accp", bufs=3) as accp,
            tc.tile_pool(name="outp", bufs=3) as outp_pool,
            tc.tile_pool(name="hps", bufs=4, space="PSUM") as hps,
            tc.tile_pool(name="scps", bufs=2, space="PSUM") as scps,
            tc.tile_pool(name="miscps", bufs=2, space="PSUM") as miscps,
        ):
            # ---- constants (small dec-path operands first: PE work sooner) ----
            w1d_sb = consts.tile([128, NKD, D], F16)
            nc.sync.dma_start(out=w1d_sb, in_=w1d_in.rearrange("a p d -> p a d"))
            b1_sb = consts.tile([1, D], F16)
            nc.sync.dma_start(out=b1_sb, in_=b1_in[:, :])
            w2_sb = consts.tile([128, NM], F16)
            nc.sync.dma_start(out=w2_sb, in_=w2_in.rearrange("a p -> p a"))
            dect_sb = consts.tile([128, NKD, BPC], F16)
            nc.sync.dma_start(out=dect_sb, in_=dec_in.rearrange("a p b -> p a b"))
            # w1e chunks interleaved with batch-0 enc tiles in queue order:
            # each arriving (weights, enc) pair feeds the next 4 matmuls of
            # the kt-outer first block, so the PE computes during the
            # initial streaming instead of waiting for the full 3.6 MB
            w1e_sb = consts.tile([128, NFT, D], F16)
            enc0_tiles = []
            for kt in range(NFT):
                nc.sync.dma_start(out=w1e_sb[:, kt, :], in_=w1e_in[kt])
                et0 = encp.tile([128, S], F16, tag="enc_t", name="enc_t")
                nc.sync.dma_start(out=et0, in_=enc_in[0, kt])
                enc0_tiles.append(et0)

            onesb = consts.tile([1, BPC], F16)
            nc.vector.memset(onesb, 1.0)
            ones16 = consts.tile([1, 128], F16)
            nc.vector.memset(ones16, 1.0)
            ones32 = consts.tile([1, 128], F32)
            nc.vector.memset(ones32, 1.0)
            shift_sb = consts.tile([1, 1], F32)
            nc.vector.memset(shift_sb, EXP_SHIFT)

            # ---- dec_proj^T + b1:  [128 (d%), NM * BPC] fp32 ----
            dec_sb = consts.tile([128, NM, BPC], F32)
            for m in range(NM):
                dp = miscps.tile([128, BPC], F32, tag="mps")
                for kd in range(NKD):
                    nc.tensor.matmul(
                        dp,
                        lhsT=w1d_sb[:, kd, m * 128:(m + 1) * 128],
                        rhs=dect_sb[:, kd, :],
                        start=(kd == 0),
                        stop=False,
                    )
                nc.tensor.matmul(
                    dp,
                    lhsT=b1_sb[0:1, m * 128:(m + 1) * 128],
                    rhs=onesb,
                    start=False,
                    stop=True,
                )
                nc.scalar.copy(dec_sb[:, m, :], dp)

            # ---- pipelined main loop ----
            defq = deque()

            def flush(n_keep: int) -> None:
                while len(defq) > n_keep:
                    defq.popleft()()

            def stage_a(sl, sb, th_tiles, e16, zacc):
                def run():
                    sc = scps.tile([1, SBLK], F32, tag="sc", name="sc")
                    for m in range(NM):
                        nc.tensor.matmul(
                            sc,
                            lhsT=w2_sb[:, m:m + 1],
                            rhs=th_tiles[m],
                            start=(m == 0),
                            stop=(m == NM - 1),
                        )
                    # e = exp(scores + EXP_SHIFT); zacc[sb] = sum(e)
                    nc.scalar.activation(
                        e16[0:1, sl], sc, Exp,
                        bias=shift_sb[0:1, 0:1],
                        accum_out=zacc[0:1, sb:sb + 1],
                    )
                return run

            def stage_b(sl, e16, ebc16):
                def run():
                    # broadcast e-chunk to all partitions (ones-matmul on PE),
                    # then park it in SBUF fp16 for the deferred weighted sum
                    ebc = miscps.tile([128, SBLK], F32, tag="mps", name="ebc")
                    nc.tensor.matmul(
                        ebc, lhsT=ones16, rhs=e16[0:1, sl], start=True, stop=True
                    )
                    nc.scalar.copy(ebc16[:, sl], ebc)
                return run

            def stage_w(b, enc_tiles, ebc16, zacc):
                def run():
                    acc2 = accp.tile([128, NFT], F32, name="acc2")
                    if abl in ("nows", "nosc", "dmaonly"):
                        nc.vector.memset(acc2, 1.0)
                    else:
                        # fused multiply + full-S free-dim reduce per tile
                        for ft in range(NFT):
                            scr = scrp.tile([128, S], F16, tag="scr",
                                            name="scr")
                            nc.vector.scalar_tensor_tensor(
                                out=scr,
                                in0=enc_tiles[ft],
                                scalar=1.0,
                                in1=ebc16,
                                op0=MUL,
                                op1=MUL,
                                accum_out=acc2[:, ft:ft + 1],
                            )
                    if abl in ("nosc", "dmaonly"):
                        o = outp_pool.tile([128, NFT], F32, name="o")
                        nc.vector.tensor_scalar_mul(o, acc2, 1.0)
                        nc.sync.dma_start(out=out_dram[b], in_=o)
                        return
                    z = zp.tile([1, 1], F32, tag="z", name="z")
                    nc.vector.tensor_reduce(z, zacc, axis=AX, op=ADD)
                    invz = zp.tile([1, 1], F32, tag="invz", name="invz")
                    nc.vector.reciprocal(invz, z)
                    izp = scps.tile([128, 1], F32, tag="sc", name="izp")
                    nc.tensor.matmul(izp, lhsT=ones32, rhs=invz,
                                     start=True, stop=True)
                    izb = zp.tile([128, 1], F32, tag="izb", name="izb")
                    nc.scalar.copy(izb, izp)
                    o = outp_pool.tile([128, NFT], F32, name="o")
                    nc.vector.tensor_scalar_mul(o, acc2, izb)
                    nc.sync.dma_start(out=out_dram[b], in_=o)
                return run

            for iter_i, b in enumerate(i % BPC for i in range(repeat * BPC)):
                if iter_i == 0:
                    enc_tiles = enc0_tiles
                else:
                    enc_tiles = []
                    for ft in range(NFT):
                        et = encp.tile([128, S], F16, tag="enc_t", name="enc_t")
                        nc.sync.dma_start(out=et, in_=enc_in[b, ft])
                        enc_tiles.append(et)

                e16 = ep_pool.tile([1, S], F16, name="e16")
                zacc = zp.tile([1, NSB], F32, tag="zacc", name="zacc")
                ebc16 = ebcp.tile([128, S], F16, tag="ebc16", name="ebc16")

                for sb in range(NSB):
                    boff = sb * SBLK
                    sl = slice(boff, boff + SBLK)
                    th_tiles = []
                    if abl == "dmaonly":
                        continue
                    if iter_i == 0 and sb == 0:
                        # kt-outer: every arriving enc tile feeds 4 matmuls,
                        # so the PE computes while the first tiles stream in
                        hp_list = [hps.tile([128, SBLK], F32, tag="hp", name="hp")
                                   for _ in range(NM)]
                        for kt in range(NFT):
                            for m in range(NM):
                                nc.tensor.matmul(
                                    hp_list[m],
                                    lhsT=w1e_sb[:, kt, m * 128:(m + 1) * 128],
                                    rhs=enc_tiles[kt][:, sl],
                                    start=(kt == 0),
                                    stop=(kt == NFT - 1),
                                )
                        for m in range(NM):
                            th = thp.tile([128, SBLK], F16, tag="th", name="th")
                            nc.scalar.activation(
                                th, hp_list[m], Tanh,
                                bias=dec_sb[:, m, b:b + 1], scale=1.0,
                            )
                            th_tiles.append(th)
                    else:
                        for m in range(NM):
                            hp = hps.tile([128, SBLK], F32, tag="hp", name="hp")
                            for kt in range(NFT):
                                nc.tensor.matmul(
                                    hp,
                                    lhsT=w1e_sb[:, kt, m * 128:(m + 1) * 128],
                                    rhs=enc_tiles[kt][:, sl],
                                    start=(kt == 0),
                                    stop=(kt == NFT - 1),
                                )
                            th = thp.tile([128, SBLK], F16, tag="th", name="th")
                            nc.scalar.activation(
                                th, hp, Tanh, bias=dec_sb[:, m, b:b + 1], scale=1.0
                            )
                            th_tiles.append(th)

                    if abl is None or abl == "nows":
                        defq.append(stage_a(sl, sb, th_tiles, e16, zacc))
                        defq.append(stage_b(sl, e16, ebc16))
                    flush(DEPTH)

                defq.append(stage_w(b, enc_tiles, ebc16, zacc))

            flush(0)

    _split_multi_waits(nc)
    return nc


def prep_inputs(encoder_output, decoder_hidden, W1, b1, W2, cfg: Cfg):
    """Host-side sharding + layout. Returns per-core input maps."""
    S, F, D, DEC, BPC = cfg.S, cfg.F, cfg.D, cfg.DEC, cfg.BPC
    NFT, NM, NKD = cfg.NFT, cfg.NM, cfg.NKD
    n_cores = encoder_output.shape[1] // BPC

    # enc [S, B, F] -> [B, F, S] -> f = p*NFT + ft -> [B, NFT, 128, S] fp16
    enc_t = np.ascontiguousarray(encoder_output.transpose(1, 2, 0))
    enc_l = np.ascontiguousarray(
        enc_t.reshape(-1, 128, NFT, S).transpose(0, 2, 1, 3)
    ).astype(np.float16)

    w1e_t = W1[:, :F].T.astype(np.float32)  # [F, D]
    w1e = np.ascontiguousarray(
        w1e_t.reshape(128, NFT, D).transpose(1, 0, 2)
    ).astype(np.float16)

    w1d_t = W1[:, F:].T.astype(np.float32)  # [DEC, D]
    w1d = np.ascontiguousarray(w1d_t.reshape(NKD, 128, D)).astype(np.float16)

    b1r = b1.reshape(1, D).astype(np.float16)
    w2t = W2.reshape(D).reshape(NM, 128).astype(np.float16)
    dect_full = decoder_hidden.T.reshape(NKD, 128, -1).astype(np.float16)  # [.., B]

    in_maps = []
    for c in range(n_cores):
        bs = slice(c * BPC, (c + 1) * BPC)
        m = {
            "enc_in": enc_l[bs],
            "w1e": w1e,
            "w1d": w1d,
            "b1r": b1r,
            "w2t": w2t,
            "dect": np.ascontiguousarray(dect_full[:, :, bs]),
        }
        in_maps.append(m)
    return in_maps


def assemble_core(r, cfg: Cfg):
    """Per-core output rows [BPC, F] from the result map."""
    return r["out"].reshape(cfg.BPC, cfg.F)


def kernel(encoder_output, decoder_hidden, W1, b1, W2, b2):
    """Full inputs in, full output out. b2 cancels in the softmax."""
    encoder_output = np.asarray(encoder_output, dtype=np.float32)
    decoder_hidden = np.asarray(decoder_hidden, dtype=np.float32)
    W1 = np.asarray(W1, dtype=np.float32)
    b1 = np.asarray(b1, dtype=np.float32)
    W2 = np.asarray(W2, dtype=np.float32)

    cfg = FULL
    nc = build_bass(cfg)
    in_maps = prep_inputs(encoder_output, decoder_hidden, W1, b1, W2, cfg)
    res = run_bass_kernel_spmd(nc, in_maps, list(range(N_CORES)))
    out = np.concatenate(
        [assemble_core(r, cfg) for r in res.results], axis=0
    )
    return out[:, None, :].astype(np.float32)


if __name__ == "__main__":
    import reference

    inputs = reference.setup_inputs()
    expected = np.asarray(reference.reference(**inputs))
    actual = kernel(**{k: np.asarray(v) for k, v in inputs.items()})
    err = np.abs(actual - expected).max() / np.abs(expected).max()
    print("Relative error:", err)
